# revision 10
# baseline (speedup 1.0000x reference)
"""Trainium2 Bass kernel for a seq2seq CandlestickLSTM.

Model (per reference): 2-layer LSTM encoder over S=64 steps, then a
2-layer LSTM decoder run autoregressively for T=32 steps with an MLP
head (Linear(H,H/2) -> ReLU -> Linear(H/2,OUT) -> Sigmoid) whose output
feeds back as the next decoder input.

Sharding: pure data parallel over 8 NeuronCores -- batch 4096 -> 512
rows per core; all weights replicated. No collectives needed.

On-core layout: feature-major ("transposed"): activations h, c live as
[128 partitions, HT, Bc] 3D tiles (hidden tile k = page k). Matmuls
compute z.T = W @ input.T via out = lhsT.T @ rhs.

fp8: all K=256 contractions (recurrent weights, Wih1, Wp1) run as
fp8e4 DoubleRow matmuls -- one [128,2,M]x[128,2,N] instruction per
gate M-tile instead of two bf16 K=128 matmuls (measured ~2x per-result
vs the bf16 pair; fp8 rel err ~2e-3 vs the 2e-2 budget, validated in
numpy). h state tensors are stored fp8; c stays bf16; the K=4 input
matmuls (x, pred) stay bf16 and are packed 4-way into 32-row
tile_position groups.

ACT: the three sigmoid gates share one PSUM tile [128,3,HT,Bc] so a
single fused ACTIVATE converts them (saves 2x352-cycle instruction
overhead per cell); tanh(g) is separate; tanh(c) is emitted per hidden
half so h[:,k,:] releases early. The decoder MLP PSUM aliases the z_g
tag (gate g is emitted last, giving the WAR dependency slack).

Dependency scheduling: within a layer-1 cell the recurrent Whh@h1
matmuls of ALL four gates are emitted first so the PE has ready work
while the layer-0 elementwise chain drains; same trick fills the
decoder's serial MLP/pred chain with the next step's Whh@h0 matmuls.
"""

import numpy as np
import ml_dtypes
from contextlib import ExitStack

import concourse.bass as bass
import concourse.tile as tile
from concourse import bacc, mybir
from concourse.bass_utils import run_bass_kernel_spmd

NCORES = 8
B, S, IN, H, OUT = 4096, 64, 4, 256, 4
BC = B // NCORES          # 512 batch rows per core
HT = H // 128             # 2 hidden 128-tiles
GT = 4 * H // 128         # 8 gate M-tiles
HH = H // 2               # 128 (MLP hidden)
F32 = mybir.dt.float32
BF16 = mybir.dt.bfloat16
F8 = mybir.dt.float8e4
AF = mybir.ActivationFunctionType
ALU = mybir.AluOpType
DR = mybir.MatmulPerfMode.DoubleRow

_BF = ml_dtypes.bfloat16
_F8 = ml_dtypes.float8_e4m3  # IEEE bias-7, max 240 -> TRN FP8_EXP4

_cache = {}


def _pack_whT(W, dt=_BF):
    """W [M, K] (acts on K-dim inputs, K%128==0) -> pack [128, K/128, M].

    [p, kt, m] = W.T[p + 128*kt, m]; the lhsT tile for (k_tile, m_tile) is
    arr[:, kt, 128m : 128m+128]. With dt=fp8 the full [:, :, mslice] is one
    DoubleRow lhsT.
    """
    M, K = W.shape
    assert K % 128 == 0
    kt = K // 128
    WT = np.ascontiguousarray(W.T)  # [K, M]
    arr = WT.reshape(kt, 128, M).transpose(1, 0, 2)
    return np.ascontiguousarray(arr).astype(dt)


def _pack_rep(W):
    """W [4H, IN] -> [128, 4H] bf16 with W.T at partition rows 32g..32g+IN-1
    for g in 0..3 (row-group replicas for tile_position-packed matmuls)."""
    rep = np.zeros((128, 4 * H), np.float32)
    for g in range(4):
        rep[32 * g : 32 * g + IN] = W.T
    return rep.astype(_BF)


def _pack_bias_rep(b):
    """b [OUT] -> [128, 1] f32 with b at rows 32g..32g+OUT-1."""
    rep = np.zeros((128, 1), np.float32)
    for g in range(4):
        rep[32 * g : 32 * g + OUT, 0] = b
    return rep


def _pack_bias(b):
    """b [4H] -> [128, GT] with column m = b[128m:128(m+1)]."""
    return np.ascontiguousarray(b.reshape(GT, 128).T).astype(np.float32)


def _build(T, lstm_bias_flags, repeats=1):
    """Build + compile the per-core program. lstm_bias_flags: 4 bools for
    (enc0, enc1, dec0, dec1) biases being nonzero."""
    nc = bacc.Bacc(
        "TRN2",
        target_bir_lowering=False,
        debug=False,
        enable_asserts=False,
    )

    def din(name, shape, dt):
        return nc.dram_tensor(name, shape, dt, kind="ExternalInput").ap()

    d_xT = din("xT", [IN, S * BC], BF16)
    d_we0x = din("we0x", [128, 4 * H], BF16)   # row-group replicated
    d_we0h = din("we0h", [128, HT, 4 * H], F8)
    d_we1x = din("we1x", [128, HT, 4 * H], F8)
    d_we1h = din("we1h", [128, HT, 4 * H], BF16)
    d_wd0x = din("wd0x", [128, 4 * H], BF16)   # row-group replicated
    d_wd0h = din("wd0h", [128, HT, 4 * H], F8)
    d_wd1x = din("wd1x", [128, HT, 4 * H], F8)
    d_wd1h = din("wd1h", [128, HT, 4 * H], BF16)
    d_wp1 = din("wp1", [128, HT, HH], BF16)
    d_wp2 = din("wp2", [HH, OUT], BF16)
    d_bp1 = din("bp1", [HH, 1], F32)
    d_bp2 = din("bp2", [128, 1], F32)          # row-group replicated
    d_lb = [None] * 4
    for li, flag in enumerate(lstm_bias_flags):
        if flag:
            d_lb[li] = din(f"lstmbias{li}", [128, GT], F32)

    out = nc.dram_tensor("out", [T, OUT, BC], F32, kind="ExternalOutput").ap()

    with tile.TileContext(nc) as tc, ExitStack() as ctx:
        persist = ctx.enter_context(tc.tile_pool(name="persist", bufs=1))

        def load(name, dram_ap, shape, dt):
            t = persist.tile(shape, dt, name=name)
            nc.sync.dma_start(t[:], dram_ap[:])
            return t

        # encoder weights first: the PE's first matmuls need we0x + x chunk 0
        s_we0x = load("s_we0x", d_we0x, [128, 4 * H], BF16)
        s_we0h = load("s_we0h", d_we0h, [128, HT, 4 * H], F8)
        s_we1x = load("s_we1x", d_we1x, [128, HT, 4 * H], F8)
        s_we1h = load("s_we1h", d_we1h, [128, HT, 4 * H], BF16)

        # x.T replicated at partition bases 0/32/64/96 (row-group replicas for
        # tile_position-packed K=IN matmuls), staged in chunks so step 0
        # doesn't wait on the whole tensor.
        s_xT = persist.tile([128, S * BC], BF16, name="s_xT")
        x_chunks = [(0, 1), (1, 8)] + [(s, s + 8) for s in range(8, S, 8)]

        def x_chunk_dma(lo, hi):
            for g in range(4):
                nc.sync.dma_start(
                    s_xT[32 * g : 32 * g + IN, BC * lo : BC * hi],
                    d_xT[:, BC * lo : BC * hi],
                )

        for lo, hi in x_chunks[:2]:
            x_chunk_dma(lo, hi)

        s_wd0x = load("s_wd0x", d_wd0x, [128, 4 * H], BF16)
        s_wd0h = load("s_wd0h", d_wd0h, [128, HT, 4 * H], F8)
        s_wd1x = load("s_wd1x", d_wd1x, [128, HT, 4 * H], F8)
        s_wd1h = load("s_wd1h", d_wd1h, [128, HT, 4 * H], BF16)
        s_wp1 = load("s_wp1", d_wp1, [128, HT, HH], BF16)
        s_wp2 = load("s_wp2", d_wp2, [HH, OUT], BF16)
        s_bp1 = load("s_bp1", d_bp1, [HH, 1], F32)
        s_bp2 = load("s_bp2", d_bp2, [128, 1], F32)
        s_lb = [None] * 4
        for li in range(4):
            if d_lb[li] is not None:
                s_lb[li] = load(f"s_lstmbias{li}", d_lb[li], [128, GT], F32)

        for lo, hi in x_chunks[2:]:
            x_chunk_dma(lo, hi)

        # zeros for the DVE relu (max with 0)
        s_zero = persist.tile([HH, BC], BF16, name="s_zero")
        nc.vector.memset(s_zero[:], 0.0)

        zp = ctx.enter_context(tc.tile_pool(name="zp", bufs=1, space="PSUM"))
        gp = ctx.enter_context(tc.tile_pool(name="gp", bufs=3))
        sp = ctx.enter_context(tc.tile_pool(name="sp", bufs=3))

        # gate order in z rows: i, f, g, o (PyTorch) -> z-row pair p.
        # Sigmoid gates (f, i, o) share one PSUM tile (page si) so one fused
        # ACTIVATE converts all three; tanh gate g has its own tile.
        # Emission order f, i, o, g: the fused sigmoid fires after o's
        # matmuls; g (and the MLP PSUM aliased on z_g) trails.
        SIGS = (("f", 1, 0), ("i", 0, 1), ("o", 3, 2))  # (name, p, page)
        P_OF = {"f": 1, "i": 0, "o": 3, "g": 2}
        EMIT_ORDER = ("f", "i", "o", "g")

        def cell(tag, layer, pre_chunks, post_chunks, x_chunk, c_prev,
                 bias_t, first, x_mode="packed_first"):
            """Emit one LSTM cell.

            pre_chunks: (w3d, h3d) fp8 K=256 DoubleRow contributions whose
              inputs are already available -- emitted up front for all
              gates (PE fill while upstream elementwise drains).
            post_chunks: contributions on freshly-produced state, per-gate
              after the pre block.
            x_chunk: None or (wx, rhs_ap) K=IN bf16 contribution, packed
              4-way into 32-row tile_position groups (wx/rhs replicated at
              partition bases 0/32/64/96). x_mode: "packed_first" (quads at
              the top -- encoder L0) | "packed_last" (quads late -- decoder,
              input is the fresh pred).
            Returns (h_new, c_new); h_new fp8, c_new bf16.
            """
            n_mm = sum(1 if dr else HT for (_, _, dr) in
                       pre_chunks + post_chunks) + (
                1 if x_chunk is not None else 0)
            z_sig = zp.tile([128, 3, HT, BC], F32, tag="z_sig", bufs=1,
                            name=f"zs_{tag}")
            z_g = zp.tile([128, HT, BC], F32, tag="z_g", bufs=1,
                          name=f"zg_{tag}")

            def zv(gname, j):
                if gname == "g":
                    return z_g[:, j, :]
                for n, p, si in SIGS:
                    if n == gname:
                        return z_sig[:, si, j, :]

            mi = {(gname, j): 0 for gname in EMIT_ORDER for j in range(HT)}

            def emit_x(gname, j, base=0, tile_pos=None):
                wx, rhs_ap = x_chunk
                key = (gname, j)
                m = 2 * P_OF[gname] + j
                nc.tensor.matmul(
                    zv(gname, j),
                    wx[base : base + IN, 128 * m : 128 * m + 128],
                    rhs_ap[base : base + IN, :],
                    start=(mi[key] == 0), stop=(mi[key] == n_mm - 1),
                    tile_position=tile_pos,
                    skip_group_check=True,
                )
                mi[key] += 1

            def emit_x_packed(quad):
                # 4 concurrent matmuls in distinct 32-row groups
                for gi, (gname, j) in enumerate(quad):
                    emit_x(gname, j, base=32 * gi, tile_pos=(32 * gi, 0))

            def emit_h(chunks, gname, j):
                key = (gname, j)
                m = 2 * P_OF[gname] + j
                for (w3, h3, dr) in chunks:
                    if dr:
                        nc.tensor.matmul(
                            zv(gname, j),
                            w3[:, :, 128 * m : 128 * m + 128],
                            h3[:, :, :],
                            start=(mi[key] == 0),
                            stop=(mi[key] == n_mm - 1),
                            perf_mode=DR,
                            skip_group_check=True,
                        )
                        mi[key] += 1
                    else:
                        for k in range(HT):
                            nc.tensor.matmul(
                                zv(gname, j),
                                w3[:, k, 128 * m : 128 * m + 128],
                                h3[:, k, :],
                                start=(mi[key] == 0),
                                stop=(mi[key] == n_mm - 1),
                                skip_group_check=True,
                            )
                            mi[key] += 1

            # pre block: x (if early) and already-available recurrent parts
            if x_chunk is not None and x_mode == "packed_first":
                emit_x_packed([("f", 0), ("f", 1), ("i", 0), ("i", 1)])
                emit_x_packed([("o", 0), ("o", 1), ("g", 0), ("g", 1)])
            for gname in EMIT_ORDER:
                for j in range(HT):
                    emit_h(pre_chunks, gname, j)

            # late x (decoder: pred arrives at the end of the prev step)
            if x_chunk is not None and x_mode == "packed_last":
                emit_x_packed([("f", 0), ("f", 1), ("i", 0), ("i", 1)])
            # per-gate fresh-state matmuls
            for gname in EMIT_ORDER:
                if gname == "o" and x_chunk is not None and \
                        x_mode == "packed_last":
                    emit_x_packed([("o", 0), ("o", 1), ("g", 0), ("g", 1)])
                for j in range(HT):
                    emit_h(post_chunks, gname, j)

            # fused activations
            g_sig = gp.tile([128, 3, HT, BC], BF16, tag="g_sig",
                            name=f"gs_{tag}")
            g_g = gp.tile([128, HT, BC], BF16, tag="gate_g", name=f"gg_{tag}")
            if bias_t is None:
                nc.scalar.activation(g_sig[:], z_sig[:], AF.Sigmoid)
                nc.scalar.activation(g_g[:], z_g[:], AF.Tanh)
            else:
                for n, p, si in SIGS:
                    for j in range(HT):
                        m = 2 * p + j
                        nc.scalar.activation(
                            g_sig[:, si, j, :], z_sig[:, si, j, :],
                            AF.Sigmoid, bias=bias_t[:, m : m + 1])
                for j in range(HT):
                    m = 2 * P_OF["g"] + j
                    nc.scalar.activation(
                        g_g[:, j, :], z_g[:, j, :], AF.Tanh,
                        bias=bias_t[:, m : m + 1])

            c_new = sp.tile([128, HT, BC], BF16, tag=f"c{layer}", name=f"c_{tag}")
            h_new = sp.tile([128, HT, BC], F8 if layer == 0 else BF16,
                            tag=f"h{layer}", name=f"h_{tag}")
            tc_t = gp.tile([128, HT, BC], BF16, tag="tanh_c", name=f"tc_{tag}")
            if not first:
                t1 = gp.tile([128, HT, BC], BF16, tag="t1", name=f"t1_{tag}")
                t2 = gp.tile([128, HT, BC], BF16, tag="t2", name=f"t2_{tag}")
            # fused chain: ACT is the bound engine, so minimize ACTIVATE
            # instruction count (352-cycle overhead each); DR consumers need
            # the full h anyway
            if first:
                nc.vector.tensor_mul(c_new[:], g_sig[:, 1, :, :], g_g[:])
            else:
                nc.vector.tensor_mul(t1[:], g_sig[:, 0, :, :], c_prev[:])
                nc.vector.tensor_mul(t2[:], g_sig[:, 1, :, :], g_g[:])
                nc.vector.tensor_add(c_new[:], t1[:], t2[:])
            nc.scalar.activation(tc_t[:], c_new[:], AF.Tanh)
            nc.vector.tensor_mul(h_new[:], g_sig[:, 2, :, :], tc_t[:])
            return h_new, c_new

        def emit_forward():
            h0 = c0 = h1 = c1 = None
            # ---------------- encoder ----------------
            for t in range(S):
                first = t == 0
                xt = s_xT[:, BC * t : BC * (t + 1)]
                # L0: x and h0(t-1) both ready at emission -> all pre
                h0, c0 = cell(
                    f"e0_{t}", 0,
                    [] if first else [(s_we0h, h0, True)], [],
                    (s_we0x, xt), c0, s_lb[0], first,
                    x_mode="packed_first",
                )
                # L1: h1(t-1) ready (pre, all gates), h0(t) fresh (post)
                h1, c1 = cell(
                    f"e1_{t}", 1,
                    [] if first else [(s_we1h, h1, False)], [(s_we1x, h0, True)],
                    None, c1, s_lb[1], first,
                )

            # ---------------- decoder ----------------
            pred_bf = None
            for t in range(T):
                if t == 0:
                    xt = s_xT[:, BC * (S - 1) : BC * S]
                else:
                    xt = pred_bf[:]
                # L0: h0(t-1) ready (pre -- fills the PE during the previous
                # step's MLP/pred chain), pred arrives late (emitted last)
                h0, c0 = cell(
                    f"d0_{t}", 0, [(s_wd0h, h0, True)], [], (s_wd0x, xt),
                    c0, s_lb[2], False,
                    x_mode="packed_last",
                )
                h1, c1 = cell(
                    f"d1_{t}", 1, [(s_wd1h, h1, False)], [(s_wd1x, h0, True)], None,
                    c1, s_lb[3], False,
                )

                # MLP head: relu(Wp1 @ h1 + bp1) -> sigmoid(Wp2 @ . + bp2).
                # PSUM aliases the z_g tag: page 0 = m1, page 1 = m2. Gate g
                # is emitted last, so the next cell's z_g matmuls have slack
                # before the WAR dependency on this tile releases.
                m12 = zp.tile([128, HT, BC], F32, tag="z_g", bufs=1,
                              name=f"m12_{t}")
                for k in range(HT):
                    nc.tensor.matmul(
                        m12[:, 0, :], s_wp1[:, k, :], h1[:, k, :],
                        start=(k == 0), stop=(k == HT - 1),
                        skip_group_check=True,
                    )
                # relu on DVE: (m1 + bp1) max 0 -- keeps ACT off the chain
                m1_sb = gp.tile([HH, BC], BF16, tag="m1sb", name=f"m1sb_{t}")
                nc.vector.scalar_tensor_tensor(
                    m1_sb[:], m12[0:HH, 0, :], s_bp1[:, 0:1], s_zero[:],
                    ALU.add, ALU.max,
                )
                # m2 as 4 concurrent col-group matmuls: pred lands at
                # partition bases 0/32/64/96 so the next step's packed L0
                # input matmuls can read their row-group replicas directly.
                for g in range(4):
                    nc.tensor.matmul(
                        m12[32 * g : 32 * g + OUT, 1, :], s_wp2[:],
                        m1_sb[:], start=True, stop=True,
                        tile_position=(0, 32 * g),
                        skip_group_check=True,
                    )
                # one sigmoid for all four replicas (partitions are free
                # parallelism); fp32 output copy comes off ACT onto DVE
                pred_bf = gp.tile([128, BC], BF16, tag="predbf",
                                  name=f"predbf_{t}")
                nc.scalar.activation(pred_bf[:], m12[:, 1, :], AF.Sigmoid,
                                     bias=s_bp2[:, 0:1])
                pred_f = gp.tile([OUT, BC], F32, tag="predf", name=f"predf_{t}")
                nc.vector.tensor_copy(pred_f[:], pred_bf[0:OUT, :])
                nc.sync.dma_start(out[t, :, :], pred_f[:])

        for _rep in range(repeats):
            emit_forward()

    nc.compile()
    return nc


def _prep_shared(inputs):
    f32 = lambda k: np.asarray(inputs[k], np.float32)
    shared = {
        "we0x": _pack_rep(f32("enc_Wih0")),
        "we0h": _pack_whT(f32("enc_Whh0"), _F8),
        "we1x": _pack_whT(f32("enc_Wih1"), _F8),
        "we1h": _pack_whT(f32("enc_Whh1")),
        "wd0x": _pack_rep(f32("dec_Wih0")),
        "wd0h": _pack_whT(f32("dec_Whh0"), _F8),
        "wd1x": _pack_whT(f32("dec_Wih1"), _F8),
        "wd1h": _pack_whT(f32("dec_Whh1")),
        "wp1": _pack_whT(f32("Wp1")),
        "wp2": np.ascontiguousarray(f32("Wp2").T).astype(_BF),
        "bp1": np.ascontiguousarray(f32("bp1").reshape(HH, 1)),
        "bp2": _pack_bias_rep(f32("bp2").reshape(OUT)),
    }
    lstm_biases = [f32("enc_b0"), f32("enc_b1"), f32("dec_b0"), f32("dec_b1")]
    flags = tuple(bool(np.any(b != 0)) for b in lstm_biases)
    for li, (b, flag) in enumerate(zip(lstm_biases, flags)):
        if flag:
            shared[f"lstmbias{li}"] = _pack_bias(b)
    return shared, flags


def _make_in_maps(inputs):
    x = np.asarray(inputs["x"], np.float32)
    assert x.shape == (B, S, IN), x.shape
    shared, _ = _prep_shared(inputs)
    in_maps = []
    for c in range(NCORES):
        xc = x[c * BC : (c + 1) * BC]                       # [BC, S, IN]
        xT = np.ascontiguousarray(xc.transpose(2, 1, 0))    # [IN, S, BC]
        in_maps.append({"xT": xT.reshape(IN, S * BC).astype(_BF), **shared})
    return in_maps


def kernel(**inputs):
    T = int(np.asarray(inputs["target_length"]))

    _, flags = _prep_shared(inputs)
    key = (T, flags)
    if key not in _cache:
        _cache[key] = _build(T, flags)
    nc = _cache[key]

    in_maps = _make_in_maps(inputs)

    res = run_bass_kernel_spmd(nc, in_maps, list(range(NCORES)))
    # per-core out is [T, OUT, BC] -> [BC, T, OUT]
    return np.concatenate(
        [res.results[i]["out"].transpose(2, 0, 1) for i in range(NCORES)],
        axis=0,
    ).astype(np.float32)
